# revision 24
# baseline (speedup 1.0000x reference)
"""Trainium2 Bass kernel for nn_CausalMultiresConv1d.

Reference computation (per batch b, channel c):
    r_0 = x
    y   = sum_{lvl=0..7} w[:, 8-lvl] * (h1 *_{d=2^lvl} r_lvl)
          + w[:,0] * r_8 + w[:,9] * x,   r_{lvl+1} = h0 *_d r_lvl
    out = gelu(y)   (exact erf gelu; causal depthwise convs, K=4 taps)

Sharding: pure data parallel — 1 batch element per NeuronCore (B=8, 8 cores).
Per-core layout: the [64ch, 32768] slice is packed as [128 partitions, 768+16384]:
  partition p = 64*j + c  ->  channel c, L-half j.
  Leading 768 cols are the causal halo: zeros for half 0, real x for half 1
  (768 >= 765 = total receptive field of the level stack), so both halves
  compute exactly with no inter-chunk communication.

Engine split (default variant hr72 — every h0 level is column-split):
  - TensorE: left ~72% of each h0-conv level as diagonal-matmul taps
    (exact fp32, 4 cyc/col) accumulating 4 shifted rhs views in PSUM.
  - ScalarE: PSUM->SBUF drains, tap-0 scaled copies of the right part,
    y init (w9*x), final exact GELU.
  - VectorE: the 32 h1 y-taps + w0-folded final taps + right-part h0 taps
    1-3 as fused scalar_tensor_tensor multiply-accumulates, with
    per-channel scalars (w (x) h1 folded on the host).
  Measured engine rates (this HW): DVE STT ~14.5us / 16K-wide pass,
  ACT pass ~13.7us, PE fp32 matmul 4 cyc/col. The 36 DVE y-chain passes
  are the capacity floor; the split ratio balances DVE vs PE.
"""

import numpy as np

import concourse.bass as bass
import concourse.mybir as mybir
from concourse.bass_utils import run_bass_kernel_spmd
from concourse.tile import TileContext
from concourse.vector_clock import ScopedClock

# The walrus build here rejects instructions carrying more than one sync-wait
# ("Too many sync wait commands"). Tile's kernel-tail drain attaches a wait for
# every outstanding semaphore to a single SP Drain. _TC splits them: hoist all
# but the last wait onto dedicated single-wait NOPs preceding the drain.


class _TC(TileContext):
    def __exit__(self, *a):
        r = super().__exit__(*a)
        _split_multi_waits(self.nc)
        return r


def _split_multi_waits(nc):
    """Post-pass: for any instruction with >1 sync waits, hoist all but the
    last onto fresh single-wait NOPs on the same engine placed just before
    it (engines execute their stream in order, so semantics are identical)."""
    n = 0
    for fn in nc.m.functions:
        for blk in fn.blocks:
            insts = getattr(blk, "instructions", None)
            if insts is None:
                continue
            new = []
            for inst in insts:
                si = getattr(inst, "sync_info", None)
                waits = list(si.on_wait) if si is not None and si.on_wait else []
                if len(waits) > 1:
                    for j, wcmd in enumerate(waits[:-1]):
                        nop = mybir.InstNoOp(
                            name=f"{inst.name}-hw{j}", engine=inst.engine
                        )
                        nop.sync_info = mybir.SyncInfo(
                            on_wait=[wcmd], on_update=[]
                        )
                        new.append(nop)
                        n += 1
                    inst.sync_info = mybir.SyncInfo(
                        on_wait=[waits[-1]], on_update=list(si.on_update)
                    )
                new.append(inst)
            blk.instructions[:] = new
    return n

B, C, L = 8, 64, 32768
K, DEPTH = 4, 8
NCORES = 8
NCHUNK = 2
CL = L // NCHUNK          # 16384 columns per chunk
PAD = 768                 # halo >= 765 = total receptive field
W = PAD + CL              # 17152 buffer columns
P = NCHUNK * C            # 128 partitions
NS = 41                   # scalar table columns
NDIAG = K * P             # 4 [128,128] diagonal weight matrices
XCOLS = W + NS + NDIAG    # total packed input columns
MMN = 512                 # matmul free-dim tile (one PSUM bank of fp32)

F32 = mybir.dt.float32
MULT = mybir.AluOpType.mult
ADD = mybir.AluOpType.add


def _build_nc(reps=1, variant="hr72"):
    """Build the per-core program. reps>1 repeats the compute phase for
    delta-based wall-clock timing (output is numerically meaningless then,
    because the conv chain scribbles over the input buffer in place)."""
    nc = bass.Bass()
    x_in = nc.dram_tensor("x", [P, XCOLS], F32, kind="ExternalInput")
    y_out = nc.dram_tensor("y", [P, CL], F32, kind="ExternalOutput")

    with _TC(nc) as tc:
        with (
            tc.tile_pool(name="main", bufs=1) as pool,
            tc.tile_pool(name="psum", bufs=6, space="PSUM") as psum_pool,
        ):
            xin = pool.tile([P, XCOLS], F32, tag="xin")
            nxt0 = pool.tile([P, W], F32, tag="nxt")
            y = pool.tile([P, CL], F32, tag="y")

            nc.sync.dma_start(out=xin[:], in_=x_in[:])
            sc = xin[:, W:W + NS]
            diag = [
                xin[:, W + NS + k * P: W + NS + (k + 1) * P] for k in range(K)
            ]

            # which h0-conv levels run on the TensorEngine (exact fp32
            # matmul, 4 cyc/col) vs ACT-tap0 + DVE-taps1..3
            hsplit = None
            if variant == "v1":
                pe_levels = set()
                act_tap0 = False
            elif variant == "v2":
                pe_levels = set(range(DEPTH - 1))
                act_tap0 = False
            elif variant.startswith("g"):
                # "g<N>": N trailing h0 levels on PE, tap0-on-ACT elsewhere
                g = int(variant[1:])
                pe_levels = set(range(DEPTH - 1 - g, DEPTH - 1))
                act_tap0 = True
            else:
                # "hr<P>": every h0 level column-split — left P% of columns
                # on PE, right part on ACT(tap0)+DVE(taps 1-3)
                hsplit = int(variant[2:]) / 100.0
                pe_levels = set()
                act_tap0 = True
            YSEG = 2  # y-pass segmentation for cross-engine pipelining

            for _rep in range(reps):
                cur = xin[:, :W]
                nxt = nxt0

                # y = w9 * x  (ACT scaled copy; Copy allows per-partition scale)
                nc.scalar.activation(
                    out=y[:], in_=cur[:, PAD:],
                    func=mybir.ActivationFunctionType.Copy,
                    scale=sc[:, 40:41],
                )

                def pe_conv(cur, nxt, d, lo, hi):
                    # h0-conv on PE over nxt columns [lo, hi)
                    o0 = lo
                    while o0 < hi:
                        nn = min(MMN, hi - o0)
                        ps = psum_pool.tile([P, MMN], F32, tag="ps")
                        for k in range(K):
                            nc.tensor.matmul(
                                ps[:, :nn],
                                lhsT=diag[k],
                                rhs=cur[:, o0 - k * d: o0 - k * d + nn],
                                start=(k == 0), stop=(k == K - 1),
                            )
                        nc.scalar.copy(out=nxt[:, o0:o0 + nn], in_=ps[:, :nn])
                        o0 += nn

                def dve_conv(cur, nxt, d, lo, hi, use_act):
                    # h0-conv via tap0 scaled-copy + 3 fused MACs, cols [lo,hi)
                    if use_act:
                        nc.scalar.activation(
                            out=nxt[:, lo:hi], in_=cur[:, lo:hi],
                            func=mybir.ActivationFunctionType.Copy,
                            scale=sc[:, 32:33],
                        )
                    else:
                        nc.vector.tensor_scalar(
                            out=nxt[:, lo:hi], in0=cur[:, lo:hi],
                            scalar1=sc[:, 32:33], scalar2=None, op0=MULT,
                        )
                    for k in (1, 2, 3):
                        nc.vector.scalar_tensor_tensor(
                            out=nxt[:, lo:hi],
                            in0=cur[:, lo - k * d: hi - k * d],
                            scalar=sc[:, 32 + k:33 + k],
                            in1=nxt[:, lo:hi],
                            op0=MULT, op1=ADD,
                        )

                def y_taps(cur, d, cols, seg=YSEG):
                    # y[:, s] += sum_k sc[col+k] * cur[:, s+PAD-k*d],
                    # segmented so downstream consumers can start early.
                    bounds = [CL * i // seg for i in range(seg + 1)]
                    for s0, s1 in zip(bounds, bounds[1:]):
                        for k in range(K):
                            nc.vector.scalar_tensor_tensor(
                                out=y[:, s0:s1],
                                in0=cur[:, PAD + s0 - k * d: PAD + s1 - k * d],
                                scalar=sc[:, cols + k:cols + k + 1],
                                in1=y[:, s0:s1],
                                op0=MULT, op1=ADD,
                            )

                V = 0  # first valid column of cur at this level
                d = 1
                for lvl in range(DEPTH):
                    last = lvl == DEPTH - 1
                    if not last:
                        # nxt = h0-conv(cur), valid from column V + 3*d.
                        # Emitted before the y-taps: the chain is the
                        # critical path, y-taps fill engine gaps.
                        start = V + 3 * d
                        if hsplit is not None:
                            mid = start + int((W - start) * hsplit)
                            mid = min(start + ((mid - start + MMN - 1) // MMN)
                                      * MMN, W)
                            pe_conv(cur, nxt, d, start, mid)
                            if mid < W:
                                dve_conv(cur, nxt, d, mid, W, act_tap0)
                        elif lvl in pe_levels:
                            pe_conv(cur, nxt, d, start, W)
                        else:
                            dve_conv(cur, nxt, d, start, W, act_tap0)
                        y_taps(cur, d, lvl * 4)
                        cur, nxt = nxt, cur
                        V = start
                    else:
                        y_taps(cur, d, lvl * 4)
                        # fold w[:,0] into the last h0 conv, accumulate into y
                        y_taps(cur, d, 36)
                    d *= 2

                # segmented gelu so the output DMA streams out as soon as
                # each segment is final
                gseg = [CL * i // 4 for i in range(5)]
                for a, b in zip(gseg, gseg[1:]):
                    nc.scalar.activation(
                        out=y[:, a:b], in_=y[:, a:b],
                        func=mybir.ActivationFunctionType.Gelu,
                    )
                    if _rep == reps - 1:
                        nc.sync.dma_start(out=y_out[:, a:b], in_=y[:, a:b])
    return nc


_NC_CACHE = {}


def _get_nc(reps=1, variant="hr72"):
    key = (reps, variant)
    if key not in _NC_CACHE:
        _NC_CACHE[key] = _build_nc(reps, variant)
    return _NC_CACHE[key]


def _scalar_table(h0, h1, w):
    """[P, NS] per-partition scalar table; partition p holds channel p % 64."""
    # lax.conv is correlation: out[l] = sum_k h[k] * x[l + (k - (K-1))*d],
    # so the tap at shift -k*d carries weight h[K-1-k].
    t = np.zeros((C, NS), np.float32)
    for lvl in range(DEPTH):
        i = DEPTH - lvl
        for k in range(K):
            t[:, lvl * 4 + k] = w[:, i] * h1[:, 0, K - 1 - k]
    for k in range(K):
        t[:, 32 + k] = h0[:, 0, K - 1 - k]
        t[:, 36 + k] = w[:, 0] * h0[:, 0, K - 1 - k]
    t[:, 40] = w[:, DEPTH + 1]
    return np.tile(t, (NCHUNK, 1))


def pack_inputs(x, h0, h1, w):
    """Host-side packing into per-core [P, XCOLS] buffers."""
    sc = _scalar_table(h0, h1, w)
    diag = np.zeros((P, NDIAG), np.float32)
    for k in range(K):
        v = np.tile(h0[:, 0, K - 1 - k], NCHUNK)
        diag[np.arange(P), k * P + np.arange(P)] = v
    in_maps = []
    for b in range(NCORES):
        buf = np.zeros((P, XCOLS), np.float32)
        for j in range(NCHUNK):
            lo = j * CL
            if lo >= PAD:
                buf[j * C:(j + 1) * C, :W] = x[b, :, lo - PAD:lo + CL]
            else:
                buf[j * C:(j + 1) * C, PAD:W] = x[b, :, lo:lo + CL]
        buf[:, W:W + NS] = sc
        buf[:, W + NS:] = diag
        in_maps.append({"x": buf})
    return in_maps


def unpack_outputs(results):
    out = np.empty((B, C, L), np.float32)
    for b, r in enumerate(results):
        yv = r["y"]
        for j in range(NCHUNK):
            out[b, :, j * CL:(j + 1) * CL] = yv[j * C:(j + 1) * C]
    return out


def kernel(x, h0, h1, w, _trace=False, _variant="hr72"):
    import os
    # the axon NTFF trace hook is unavailable here; make sure a stray
    # BASS_TRACE in the environment can't break execution
    os.environ.setdefault("BASS_NEVER_TRACE", "1")

    x = np.asarray(x, np.float32)
    h0 = np.asarray(h0, np.float32)
    h1 = np.asarray(h1, np.float32)
    w = np.asarray(w, np.float32)

    in_maps = pack_inputs(x, h0, h1, w)
    nc = _get_nc(1, _variant)
    try:
        res = run_bass_kernel_spmd(
            nc, in_maps, core_ids=list(range(NCORES)), trace=_trace,
        )
    except Exception:
        # transient "device unrecoverable" failures have been observed on
        # this fleet; one retry usually succeeds
        res = run_bass_kernel_spmd(
            nc, in_maps, core_ids=list(range(NCORES)), trace=_trace,
        )
    out = unpack_outputs(res.results)
    if _trace:
        return out, res
    return out
